# revision 9
# baseline (speedup 1.0000x reference)
"""Trainium2 Bass kernel for a 2-layer bidirectional GRU + linear head.

Problem: B=64, S=4096, D_IN=7, H=128, PyTorch gate order (r, z, n).

Sharding: SEQUENCE-parallel across 8 NeuronCores. Each core owns a 512-step
segment and processes the full batch (Bc=64). Correctness across segment
boundaries uses the GRU's fast state forgetting (measured contraction:
cold-start divergence < 1e-13 after 64 steps): each scan runs over a
[s0-2T, s1+T) window (T=64). Outside [0, S) the input is replaced by "null
steps" that drive the r and z gates to sigmoid(-30) ~= 0 and the n-gate
x-term to 0, which maps any h to ~0 in one step — so edge cores start their
scans with exactly h=0 at the sequence boundary, and out-of-range halo data
feeds ~0 into layer 1.

Per-core layout (all f32 PSUM accumulation, bf16 elsewhere for FWL weight
loads and 2x DVE):
  - chunks of C=4 steps; rz gate projections prefilled into a 2-bank PSUM
    tile by bulk matmuls (biases ride extra rows of the x tensor at layer 0
    and a [4, 2W] be/ne tensor at layer 1); per-step W_hh matmuls accumulate
    on top, so sigmoid reads (x+h) terms straight from PSUM.
  - the n-gate W_hh product accumulates into a per-chunk PSUM tile whose
    b_hh_n bias is prestaged with one rank-2 matmul per chunk.
  - both directions are packed into the free dim (cols 0:64 fwd, 64:128 bwd
    of every elementwise op); the backward direction consumes a
    host-reversed x copy so all its tensors are in scan order.
  - sigma(r) and sigma(z) are separate ACT ops: sigma(z) is off the serial
    h-chain (only needed at the final update), shortening the per-step
    critical path.
"""

import numpy as np

import concourse.bass as bass
import concourse.tile as tile
from concourse import bacc, mybir
from concourse.bass import ds

F32 = mybir.dt.float32
BF = mybir.dt.bfloat16
AF = mybir.ActivationFunctionType

H = 128
DIN = 7
B = 64
S = 4096
NCORES = 8

T = 64                  # burn-in halo steps
SL = S // NCORES        # 512 owned steps per core
NSTEP = SL + 3 * T      # 704 scan steps per direction per layer
Bc = B                  # full batch per core
C = 4                   # steps per chunk
W = C * Bc              # 256 chunk columns
NCH = NSTEP // C        # 176 chunks
UNROLL = 16             # chunks per For_i iteration
NITER = NCH // UNROLL   # 11
CH = 8                  # head steps per chunk
NULL = -30.0


def build_program(npass=1):
    """npass > 1 repeats the whole computation in an outer hardware loop —
    used only by the benchmark harness to amortize dispatch overhead."""
    nc = bacc.Bacc("TRN2", target_bir_lowering=False, debug=False)

    # ---- DRAM I/O ----
    xf = nc.dram_tensor("xf", [DIN + 2, NSTEP * Bc], BF, kind="ExternalInput").ap()
    xr = nc.dram_tensor("xr", [DIN + 2, NSTEP * Bc], BF, kind="ExternalInput").ap()
    whhT = nc.dram_tensor("whhT", [12, H, H], BF, kind="ExternalInput").ap()
    wih0T = nc.dram_tensor("wih0T", [2, DIN + 2, 3 * H], BF, kind="ExternalInput").ap()
    wih1T = nc.dram_tensor("wih1T", [2, 2, H, 3 * H], BF, kind="ExternalInput").ap()
    bias1q = nc.dram_tensor("bias1q", [3, 4, H], BF, kind="ExternalInput").ap()
    bene4 = nc.dram_tensor("bene4", [4, NCH * 2 * W], BF, kind="ExternalInput").ap()
    bhhnq = nc.dram_tensor("bhhnq", [2, 2, H], BF, kind="ExternalInput").ap()
    selq = nc.dram_tensor("selq", [2, C * 2 * Bc], BF, kind="ExternalInput").ap()
    woutp = nc.dram_tensor("woutp", [H, 2], BF, kind="ExternalInput").ap()
    boutp = nc.dram_tensor("boutp", [1, 1], F32, kind="ExternalInput").ap()
    out = nc.dram_tensor("out", [SL, Bc], F32, kind="ExternalOutput").ap()
    out_flat = out.rearrange("s b -> (s b)")

    with tile.TileContext(nc) as tc:
        from contextlib import ExitStack

        stack = ExitStack()
        consts = stack.enter_context(tc.tile_pool(name="consts", bufs=1))

        # ---- persistent SBUF constants ----
        whh_sb = consts.tile([H, 12 * H], BF)  # (l,d,g) blocks of 128 cols
        for k in range(12):
            nc.sync.dma_start(whh_sb[:, k * H:(k + 1) * H], whhT[k])
        wih0_sb = consts.tile([DIN + 2, 2 * 3 * H], BF)
        for d in range(2):
            nc.sync.dma_start(wih0_sb[:, d * 3 * H:(d + 1) * 3 * H], wih0T[d])
        wih1_sb = consts.tile([H, 4 * 3 * H], BF)  # (d,k) blocks of 384 cols
        for d in range(2):
            for k in range(2):
                c0 = (d * 2 + k) * 3 * H
                nc.sync.dma_start(wih1_sb[:, c0:c0 + 3 * H], wih1T[d, k])
        bias1q_sb = consts.tile([4, 3 * H], BF)  # gate-major blocks
        for g in range(3):
            nc.sync.dma_start(bias1q_sb[:, g * H:(g + 1) * H], bias1q[g])
        bhhn_sb = consts.tile([2, 2 * H], BF)  # [row, layer*128+col]
        for l in range(2):
            nc.sync.dma_start(bhhn_sb[:, l * H:(l + 1) * H], bhhnq[l])
        selq_sb = consts.tile([2, C * 2 * Bc], BF)
        nc.sync.dma_start(selq_sb[:], selq[:])
        wout_sb = consts.tile([H, 2], BF)
        nc.sync.dma_start(wout_sb[:], woutp[:])
        bout_sb = consts.tile([1, 1], F32)
        nc.sync.dma_start(bout_sb[:], boutp[:])
        hstate = consts.tile([H, 2 * Bc], BF)

        # ---- internal DRAM: layer outputs (backward dir in scan order) ----
        h0f = nc.dram_tensor("h0f", [H, NSTEP, Bc], BF, kind="Internal").ap()
        h0b = nc.dram_tensor("h0b", [H, NSTEP, Bc], BF, kind="Internal").ap()
        h1f = nc.dram_tensor("h1f", [H, NSTEP, Bc], BF, kind="Internal").ap()
        h1b = nc.dram_tensor("h1b", [H, NSTEP, Bc], BF, kind="Internal").ap()

        def whh(l, d, g):
            k = (l * 2 + d) * 3 + g
            return whh_sb[:, k * H:(k + 1) * H]

        pools = {}

        def open_pools(rec):
            pools["rhsp"] = rec.enter_context(tc.tile_pool(name="rhsp", bufs=3))
            pools["gxnp"] = rec.enter_context(tc.tile_pool(name="gxnp", bufs=2))
            pools["ringp"] = rec.enter_context(tc.tile_pool(name="ringp", bufs=2))
            pools["stepp"] = rec.enter_context(tc.tile_pool(name="stepp", bufs=3))
            pools["psp"] = rec.enter_context(tc.tile_pool(name="psp", bufs=2, space="PSUM"))
            pools["psnp"] = rec.enter_context(tc.tile_pool(name="psnp", bufs=2, space="PSUM"))
            pools["njp"] = rec.enter_context(tc.tile_pool(name="njp", bufs=2, space="PSUM"))

        def emit_step(l, j, ring, h_prev, rz, psn, gxn):
            stepp = pools["stepp"]
            js = slice(j * Bc, (j + 1) * Bc)
            hf, hb = h_prev[:, 0:Bc], h_prev[:, Bc:2 * Bc]
            last = j == C - 1
            # serial-path matmuls first: r gates, then n-gate accumulate
            nc.tensor.matmul(rz[:, 0, js], whh(l, 0, 0), hf,
                             start=False, stop=False, skip_group_check=True)
            nc.tensor.matmul(rz[:, 1, js], whh(l, 1, 0), hb,
                             start=False, stop=last, skip_group_check=True)
            nc.tensor.matmul(psn[:, j, 0:Bc], whh(l, 0, 2), hf,
                             start=False, stop=False, skip_group_check=True)
            nc.tensor.matmul(psn[:, j, Bc:2 * Bc], whh(l, 1, 2), hb,
                             start=False, stop=last, skip_group_check=True)
            # z gates are off the serial path (consumed only at the update)
            nc.tensor.matmul(rz[:, 2, js], whh(l, 0, 1), hf,
                             start=False, stop=False, skip_group_check=True)
            nc.tensor.matmul(rz[:, 3, js], whh(l, 1, 1), hb,
                             start=False, stop=last, skip_group_check=True)

            r_t = stepp.tile([H, 2, Bc], BF, tag="r")
            nc.scalar.activation(r_t[:], rz[:, 0:2, js], AF.Sigmoid)
            z_t = stepp.tile([H, 2, Bc], BF, tag="z")
            nc.scalar.activation(z_t[:], rz[:, 2:4, js], AF.Sigmoid)
            rn = stepp.tile([H, 2 * Bc], BF, tag="rn")
            nc.vector.tensor_mul(rn[:], r_t[:], psn[:, j, :])
            arg = stepp.tile([H, 2 * Bc], BF, tag="arg")
            nc.vector.tensor_add(arg[:], rn[:], gxn[:, :, js])
            n_t = stepp.tile([H, 2 * Bc], BF, tag="n")
            nc.scalar.activation(n_t[:], arg[:], AF.Tanh)
            d_t = stepp.tile([H, 2 * Bc], BF, tag="d")
            nc.vector.tensor_sub(d_t[:], h_prev, n_t[:])
            zd = stepp.tile([H, 2 * Bc], BF, tag="zd")
            nc.vector.tensor_mul(zd[:], z_t[:], d_t[:])
            nc.vector.tensor_add(ring[:, j, :], n_t[:], zd[:])

        def emit_chunk(l, i, u, prev_ring):
            """One C-step chunk. i = For_i loop var, u = python unroll index.
            prev_ring: AP [H, 2Bc] holding h state entering the chunk."""
            cw = i * UNROLL * W + u * W        # column offset into xf/xr
            cc = i * UNROLL * C + u * C        # chunk offset into h rings
            rhsp = pools["rhsp"]
            rz = pools["psp"].tile([H, 4, W], F32, tag="rz")
            psn = pools["psnp"].tile([H, C, 2 * Bc], F32, tag="psn")
            nj = pools["njp"].tile([H, 2, W], F32, tag="nj")
            gxn = pools["gxnp"].tile([H, 2, W], BF, tag="gxn")
            ring = pools["ringp"].tile([H, C, 2 * Bc], BF, tag="ring")

            # prestage b_hh_n into the psn tile (whole bank, start=True)
            nc.tensor.matmul(psn[:], bhhn_sb[:, l * H:(l + 1) * H], selq_sb[:],
                             start=True, stop=False, skip_group_check=True)

            if l == 0:
                xf_ch = rhsp.tile([DIN + 2, W], BF, tag="xf")
                nc.sync.dma_start(xf_ch[:], xf[:, ds(cw, W)])
                xr_ch = rhsp.tile([DIN + 2, W], BF, tag="xr")
                nc.sync.dma_start(xr_ch[:], xr[:, ds(cw, W)])
                for dd, src in enumerate((xf_ch, xr_ch)):
                    b0 = dd * 3 * H
                    nc.tensor.matmul(rz[:, dd, :], wih0_sb[:, b0:b0 + H],
                                     src[:], start=(dd == 0), stop=False,
                                     skip_group_check=True)
                    nc.tensor.matmul(rz[:, 2 + dd, :], wih0_sb[:, b0 + H:b0 + 2 * H],
                                     src[:], start=(dd == 0), stop=False,
                                     skip_group_check=True)
                    nc.tensor.matmul(nj[:, dd, :], wih0_sb[:, b0 + 2 * H:b0 + 3 * H],
                                     src[:], start=(dd == 0), stop=(dd == 1),
                                     skip_group_check=True)
            else:
                ff = rhsp.tile([H, C, Bc], BF, tag="ff")
                nc.sync.dma_start(ff[:], h0f[:, ds(cc, C), :])
                brv = rhsp.tile([H, C, Bc], BF, tag="brv")
                nc.sync.dma_start(brv[:, ::-1, :],
                                  h0b[:, ds(NSTEP - C - cc, C), :])
                frv = rhsp.tile([H, C, Bc], BF, tag="frv")
                nc.sync.dma_start(frv[:, ::-1, :],
                                  h0f[:, ds(NSTEP - C - cc, C), :])
                bb = rhsp.tile([H, C, Bc], BF, tag="bb")
                nc.sync.dma_start(bb[:], h0b[:, ds(cc, C), :])
                bene = rhsp.tile([4, 2 * W], BF, tag="bene")
                nc.sync.dma_start(bene[:], bene4[:, ds(2 * cw, 2 * W)])
                # combined bias+null prefill, one matmul per PSUM bank
                nc.tensor.matmul(rz[:, 0:2, :], bias1q_sb[:, 0:H], bene[:],
                                 start=True, stop=False, skip_group_check=True)
                nc.tensor.matmul(rz[:, 2:4, :], bias1q_sb[:, H:2 * H], bene[:],
                                 start=True, stop=False, skip_group_check=True)
                nc.tensor.matmul(nj[:], bias1q_sb[:, 2 * H:3 * H], bene[:],
                                 start=True, stop=False, skip_group_check=True)
                for dd, (rA, rB) in enumerate(((ff, brv), (frv, bb))):
                    b0 = dd * 2 * 3 * H
                    for g, dst in ((0, rz[:, dd, :]), (1, rz[:, 2 + dd, :])):
                        nc.tensor.matmul(dst, wih1_sb[:, b0 + g * H:b0 + (g + 1) * H],
                                         rA[:], start=False, stop=False,
                                         skip_group_check=True)
                        nc.tensor.matmul(dst, wih1_sb[:, b0 + 3 * H + g * H:b0 + 3 * H + (g + 1) * H],
                                         rB[:], start=False, stop=False,
                                         skip_group_check=True)
                    nc.tensor.matmul(nj[:, dd, :], wih1_sb[:, b0 + 2 * H:b0 + 3 * H],
                                     rA[:], start=False, stop=False,
                                     skip_group_check=True)
                    nc.tensor.matmul(nj[:, dd, :], wih1_sb[:, b0 + 3 * H + 2 * H:b0 + 3 * H + 3 * H],
                                     rB[:], start=False, stop=(dd == 1),
                                     skip_group_check=True)

            # n-gate gx: PSUM -> bf16 SBUF ring (split across DVE and ACT)
            nc.vector.tensor_copy(gxn[:, 0, :], nj[:, 0, :])
            nc.scalar.copy(gxn[:, 1, :], nj[:, 1, :])

            for j in range(C):
                h_prev = prev_ring if j == 0 else ring[:, j - 1, :]
                emit_step(l, j, ring, h_prev, rz, psn, gxn)

            h_f_dst, h_b_dst = (h0f, h0b) if l == 0 else (h1f, h1b)
            nc.sync.dma_start(h_f_dst[:, ds(cc, C), :], ring[:, :, 0:Bc])
            nc.sync.dma_start(h_b_dst[:, ds(cc, C), :], ring[:, :, Bc:2 * Bc])
            return ring

        from contextlib import nullcontext

        hints = (mybir.EngineType.PE, mybir.EngineType.DVE)
        outer = (tc.For_i(0, npass, 1, name="npass", hint_engines=hints)
                 if npass > 1 else nullcontext())
        with outer:
            rec = ExitStack()
            open_pools(rec)
            for l in range(2):
                nc.vector.memset(hstate[:], 0.0)
                with tc.For_i(0, NITER, 1, name=f"layer{l}", hint_engines=hints) as i:
                    prev = hstate[:]
                    for u in range(UNROLL):
                        ring = emit_chunk(l, i, u, prev)
                        prev = ring[:, C - 1, :]
                    nc.vector.tensor_copy(hstate[:], prev)
            rec.close()

            # ---- head: out[r, b] = wout_f . f1[r] + wout_b . b1[r] + bout ----
            with tc.tile_pool(name="headp", bufs=3) as hp, \
                 tc.tile_pool(name="headps", bufs=2, space="PSUM") as hps:
                for k in range(SL // CH):
                    fch = hp.tile([H, CH, Bc], BF, tag="fch")
                    nc.sync.dma_start(fch[:], h1f[:, 2 * T + k * CH:2 * T + (k + 1) * CH, :])
                    bch = hp.tile([H, CH, Bc], BF, tag="bch")
                    u0 = T + SL - (k + 1) * CH
                    nc.sync.dma_start(bch[:, ::-1, :], h1b[:, u0:u0 + CH, :])
                    pso = hps.tile([1, CH * Bc], F32, tag="pso")
                    nc.tensor.matmul(pso[:], wout_sb[:, 0:1], fch[:],
                                     start=True, stop=False, skip_group_check=True)
                    nc.tensor.matmul(pso[:], wout_sb[:, 1:2], bch[:],
                                     start=False, stop=True, skip_group_check=True)
                    osb = hp.tile([1, CH * Bc], F32, tag="osb")
                    nc.scalar.activation(osb[:], pso[:], AF.Identity,
                                         bias=bout_sb[0:1, 0:1])
                    nc.sync.dma_start(out_flat[k * CH * Bc:(k + 1) * CH * Bc], osb[:])
        stack.close()

    nc.compile()
    return nc


_PROGRAM_CACHE = {}


def _get_program(npass=1):
    if npass not in _PROGRAM_CACHE:
        _PROGRAM_CACHE[npass] = build_program(npass)
    return _PROGRAM_CACHE[npass]


def _pack_host_inputs(inputs):
    """Build the per-core input maps from the full problem inputs."""
    from ml_dtypes import bfloat16
    x = np.asarray(inputs["x"], np.float32)

    # padded input over t in [-2T, S+2T): 7 x-rows + bias-enable + null-enable
    PADT = S + 4 * T
    xpad = np.zeros((DIN + 2, PADT, B), np.float32)
    xpad[:DIN, 2 * T:2 * T + S] = x.transpose(2, 1, 0)
    xpad[DIN, 2 * T:2 * T + S] = 1.0
    xpad[DIN + 1, :2 * T] = 1.0
    xpad[DIN + 1, 2 * T + S:] = 1.0

    def gT(w, g):
        return np.asarray(w, np.float32)[g * H:(g + 1) * H].T

    whhT = np.stack([
        gT(inputs[f"whh{l}{d}"], g)
        for l in range(2) for d in "fb" for g in range(3)
    ]).astype(bfloat16)

    wih0T = np.zeros((2, DIN + 2, 3 * H), np.float32)
    for di, d in enumerate("fb"):
        wih = np.asarray(inputs[f"wih0{d}"], np.float32)
        bih = np.asarray(inputs[f"bih0{d}"], np.float32)
        bhh = np.asarray(inputs[f"bhh0{d}"], np.float32)
        wih0T[di, :DIN] = wih.T
        bias = bih.copy()
        bias[:2 * H] += bhh[:2 * H]
        wih0T[di, DIN] = bias
        wih0T[di, DIN + 1, :2 * H] = NULL

    wih1T = np.zeros((2, 2, H, 3 * H), np.float32)
    bias1q = np.zeros((3, 4, H), np.float32)
    bhhnq = np.zeros((2, 2, H), np.float32)
    for di, d in enumerate("fb"):
        wih = np.asarray(inputs[f"wih1{d}"], np.float32)
        bih = np.asarray(inputs[f"bih1{d}"], np.float32)
        bhh = np.asarray(inputs[f"bhh1{d}"], np.float32)
        for k in range(2):
            for g in range(3):
                wih1T[di, k, :, g * H:(g + 1) * H] = \
                    wih[g * H:(g + 1) * H, k * H:(k + 1) * H].T
        for g in range(3):
            bias = bih[g * H:(g + 1) * H].copy()
            if g < 2:
                bias += bhh[g * H:(g + 1) * H]
            bias1q[g, 2 * di] = bias
            if g < 2:
                bias1q[g, 2 * di + 1] = NULL
        bhhnq[1, di] = bhh[2 * H:]
        bhh0 = np.asarray(inputs[f"bhh0{d}"], np.float32)
        bhhnq[0, di] = bhh0[2 * H:]

    selq = np.zeros((2, C * 2 * Bc), np.float32)
    cols = np.arange(C * 2 * Bc)
    selq[0, (cols % (2 * Bc)) < Bc] = 1.0
    selq[1, (cols % (2 * Bc)) >= Bc] = 1.0

    woutp = np.zeros((H, 2), np.float32)
    wout = np.asarray(inputs["wout"], np.float32)
    woutp[:, 0] = wout[0, :H]
    woutp[:, 1] = wout[0, H:]
    boutp = np.asarray(inputs["bout"], np.float32).reshape(1, 1)

    shared = dict(
        whhT=whhT,
        wih0T=wih0T.astype(bfloat16),
        wih1T=wih1T.astype(bfloat16),
        bias1q=bias1q.astype(bfloat16),
        bhhnq=bhhnq.astype(bfloat16),
        selq=selq.astype(bfloat16),
        woutp=woutp.astype(bfloat16),
        boutp=boutp,
    )

    in_maps = []
    for c in range(NCORES):
        win = xpad[:, SL * c:SL * c + NSTEP, :]          # t = s0-2T .. s1+T
        xf_c = np.ascontiguousarray(win.reshape(DIN + 2, NSTEP * Bc))
        xr_c = np.ascontiguousarray(win[:, ::-1, :].reshape(DIN + 2, NSTEP * Bc))
        bene = np.zeros((4, NCH, 2 * W), np.float32)
        bene[0:2, :, 0:W] = xf_c[DIN:DIN + 2].reshape(2, NCH, W)
        bene[2:4, :, W:2 * W] = xr_c[DIN:DIN + 2].reshape(2, NCH, W)
        in_maps.append(dict(
            shared,
            xf=xf_c.astype(bfloat16),
            xr=xr_c.astype(bfloat16),
            bene4=np.ascontiguousarray(bene.reshape(4, NCH * 2 * W)).astype(bfloat16),
        ))
    return in_maps


def _assemble_output(results) -> np.ndarray:
    outs = [r["out"] for r in results]  # each [SL, Bc]
    return np.concatenate(outs, axis=0).T.astype(np.float32)


def kernel(**inputs) -> np.ndarray:
    from concourse import bass_utils
    nc = _get_program()
    in_maps = _pack_host_inputs(inputs)
    res = bass_utils.run_bass_kernel_spmd(nc, in_maps, core_ids=list(range(NCORES)))
    return _assemble_output(res.results)


# revision 14
# speedup vs baseline: 2.5352x; 2.5352x over previous
"""Trainium2 Bass kernel for a 2-layer bidirectional GRU + linear head.

Problem: B=64, S=4096, D_IN=7, H=128, PyTorch gate order (r, z, n).

Sharding: SEQUENCE-parallel, 16 segments of 256 steps across 8 NeuronCores —
each core interleaves FOUR independent segment chains (G=4) through the same
instruction stream, quartering the serial step count; fixed per-instruction
overheads amortize over 2x wider ops. Boundary correctness uses the GRU's
fast forgetting (cold-start divergence < 1e-7 within 32 steps): every scan
runs a [seg_start-2T, seg_end+T) window (T=32) and out-of-range steps are
"null steps" (r,z driven to sigmoid(-30)~=0, n-gate x-term 0) that reset h
to ~0 in one step.

Per fat-step (both dirs x both segments packed in the free dim):
  6 PE matmuls (r,z,n gates x 2 dirs, N=128), one merged sigmoid over all
  r/z slices, one tanh, and 6 DVE ops; the update uses
  h' = (1-z).n + z.h with (1-z) and z.h computed while tanh runs, leaving
  only two DVE ops after tanh on the serial chain.
"""

import os

import numpy as np

import concourse.bass as bass
import concourse.tile as tile
from concourse import bacc, mybir
from concourse.bass import ds

F32 = mybir.dt.float32
BF = mybir.dt.bfloat16
AF = mybir.ActivationFunctionType
ALU = mybir.AluOpType

H = 128
DIN = 7
B = 64
S = 4096
NCORES = 8

G = 4                   # segment chains per core
SEGL = S // (NCORES * G)  # 128 owned steps per segment
T = 16                  # burn-in halo steps
NSTEP = SEGL + 3 * T    # 352 scan steps per direction per layer
Bc = B                  # full batch per chain
FAT = G * Bc            # 128 packed cols per direction
C = 1                   # fat-steps per chunk
W = C * FAT             # 256 chunk columns per direction
NCH = NSTEP // C        # 176 chunks
UNROLL = 16             # chunks per For_i iteration
NITER = NCH // UNROLL   # 11
CH = 8                  # head steps per chunk
NULL = -30.0
ABL = os.environ.get("ABL", "")


def build_program(npass=1):
    nc = bacc.Bacc("TRN2", target_bir_lowering=False, debug=False)

    # ---- DRAM I/O ----
    # xfr packs the forward- and reverse-order x chunks side by side so one
    # DMA per chunk fetches both directions: [9, chunk, {fwd,rev}, W]
    xfr = nc.dram_tensor("xfr", [DIN + 2, NCH * 2 * W], BF, kind="ExternalInput").ap()
    whhT = nc.dram_tensor("whhT", [12, H, H], BF, kind="ExternalInput").ap()
    wih0T = nc.dram_tensor("wih0T", [2, DIN + 2, 3 * H], BF, kind="ExternalInput").ap()
    wih1T = nc.dram_tensor("wih1T", [2, 2, H, 3 * H], BF, kind="ExternalInput").ap()
    bias1q = nc.dram_tensor("bias1q", [3, 4, H], BF, kind="ExternalInput").ap()
    bene4 = nc.dram_tensor("bene4", [4, NCH * 2 * W], BF, kind="ExternalInput").ap()
    bhhnq = nc.dram_tensor("bhhnq", [2, 2, H], BF, kind="ExternalInput").ap()
    selq = nc.dram_tensor("selq", [2, C * 2 * FAT], BF, kind="ExternalInput").ap()
    woutp = nc.dram_tensor("woutp", [H, 2], BF, kind="ExternalInput").ap()
    boutp = nc.dram_tensor("boutp", [1, 1], F32, kind="ExternalInput").ap()
    out = nc.dram_tensor("out", [G * SEGL, Bc], F32, kind="ExternalOutput").ap()
    out_flat = out.rearrange("s b -> (s b)")

    with tile.TileContext(nc) as tc:
        from contextlib import ExitStack, nullcontext

        stack = ExitStack()
        consts = stack.enter_context(tc.tile_pool(name="consts", bufs=1))

        # ---- persistent SBUF constants ----
        whh_sb = consts.tile([H, 12 * H], BF)  # (l,d,g) blocks of 128 cols
        for k in range(12):
            nc.sync.dma_start(whh_sb[:, k * H:(k + 1) * H], whhT[k])
        wih0_sb = consts.tile([DIN + 2, 2 * 3 * H], BF)
        for d in range(2):
            nc.sync.dma_start(wih0_sb[:, d * 3 * H:(d + 1) * 3 * H], wih0T[d])
        wih1_sb = consts.tile([H, 4 * 3 * H], BF)  # (d,k) blocks of 384 cols
        for d in range(2):
            for k in range(2):
                c0 = (d * 2 + k) * 3 * H
                nc.sync.dma_start(wih1_sb[:, c0:c0 + 3 * H], wih1T[d, k])
        bias1q_sb = consts.tile([4, 3 * H], BF)  # gate-major blocks
        for g in range(3):
            nc.sync.dma_start(bias1q_sb[:, g * H:(g + 1) * H], bias1q[g])
        bhhn_sb = consts.tile([2, 2 * H], BF)  # [row, layer*128+col]
        for l in range(2):
            nc.sync.dma_start(bhhn_sb[:, l * H:(l + 1) * H], bhhnq[l])
        selq_sb = consts.tile([2, C * 2 * FAT], BF)
        nc.sync.dma_start(selq_sb[:], selq[:])
        wout_sb = consts.tile([H, 2], BF)
        nc.sync.dma_start(wout_sb[:], woutp[:])
        bout_sb = consts.tile([1, 1], F32)
        nc.sync.dma_start(bout_sb[:], boutp[:])
        hstate = consts.tile([H, 2 * FAT], BF)

        # ---- internal DRAM: layer outputs, both dirs side by side ----
        # cols = [fwd FAT | bwd FAT]; bwd is in scan (reversed) order
        h0c = nc.dram_tensor("h0c", [H, NSTEP, 2 * FAT], BF, kind="Internal").ap()
        h1c = nc.dram_tensor("h1c", [H, NSTEP, 2 * FAT], BF, kind="Internal").ap()

        def whh(l, d, g):
            k = (l * 2 + d) * 3 + g
            return whh_sb[:, k * H:(k + 1) * H]

        pools = {}

        def open_pools(rec):
            pools["rhsp"] = rec.enter_context(tc.tile_pool(name="rhsp", bufs=3))
            pools["gxnp"] = rec.enter_context(tc.tile_pool(name="gxnp", bufs=2))
            pools["ringp"] = rec.enter_context(tc.tile_pool(name="ringp", bufs=2))
            pools["stepp"] = rec.enter_context(tc.tile_pool(name="stepp", bufs=3))
            pools["psp"] = rec.enter_context(tc.tile_pool(name="psp", bufs=2, space="PSUM"))
            pools["psnp"] = rec.enter_context(tc.tile_pool(name="psnp", bufs=2, space="PSUM"))
            pools["njp"] = rec.enter_context(tc.tile_pool(name="njp", bufs=2, space="PSUM"))

        def emit_step(l, j, ring, h_prev, rz, psn, gxn):
            stepp = pools["stepp"]
            js = slice(j * FAT, (j + 1) * FAT)
            hf, hb = h_prev[:, 0:FAT], h_prev[:, FAT:2 * FAT]
            last = j == C - 1
            # serial-path matmuls first: r gates, then n, then z
            nc.tensor.matmul(rz[:, 0, js], whh(l, 0, 0), hf,
                             start=False, stop=False, skip_group_check=True)
            nc.tensor.matmul(rz[:, 1, js], whh(l, 1, 0), hb,
                             start=False, stop=last, skip_group_check=True)
            nc.tensor.matmul(psn[:, j, 0:FAT], whh(l, 0, 2), hf,
                             start=False, stop=False, skip_group_check=True)
            nc.tensor.matmul(psn[:, j, FAT:2 * FAT], whh(l, 1, 2), hb,
                             start=False, stop=last, skip_group_check=True)
            nc.tensor.matmul(rz[:, 2, js], whh(l, 0, 1), hf,
                             start=False, stop=False, skip_group_check=True)
            nc.tensor.matmul(rz[:, 3, js], whh(l, 1, 1), hb,
                             start=False, stop=last, skip_group_check=True)

            srz = stepp.tile([H, 4, FAT], BF, tag="srz")
            nc.scalar.activation(srz[:], rz[:, :, js], AF.Sigmoid)
            rn = stepp.tile([H, 2 * FAT], BF, tag="rn")
            nc.vector.tensor_mul(rn[:], srz[:, 0:2, :], psn[:, j, :])
            arg = stepp.tile([H, 2 * FAT], BF, tag="arg")
            nc.vector.tensor_add(arg[:], rn[:], gxn[:, :, js])
            # omz/zh execute on DVE while tanh runs on ACT
            omz = stepp.tile([H, 2 * FAT], BF, tag="omz")
            nc.vector.tensor_scalar(omz[:], srz[:, 2:4, :], -1.0, 1.0,
                                    ALU.mult, ALU.add)
            zh = stepp.tile([H, 2 * FAT], BF, tag="zh")
            nc.vector.tensor_mul(zh[:], srz[:, 2:4, :], h_prev)
            n_t = stepp.tile([H, 2 * FAT], BF, tag="n")
            nc.scalar.activation(n_t[:], arg[:], AF.Tanh)
            omzn = stepp.tile([H, 2 * FAT], BF, tag="omzn")
            nc.vector.tensor_mul(omzn[:], omz[:], n_t[:])
            nc.vector.tensor_add(ring[:, j, :], omzn[:], zh[:])

        def emit_chunk(l, i, u, prev_ring):
            cw = i * UNROLL * W + u * W        # column offset into xf/xr
            cc = i * UNROLL * C + u * C        # chunk offset into h rings
            rhsp = pools["rhsp"]
            rz = pools["psp"].tile([H, 4, W], F32, tag="rz")
            psn = pools["psnp"].tile([H, C, 2 * FAT], F32, tag="psn")
            nj = pools["njp"].tile([H, 2, W], F32, tag="nj")
            gxn = pools["gxnp"].tile([H, 2, W], BF, tag="gxn")
            ring = pools["ringp"].tile([H, C, 2 * FAT], BF, tag="ring")

            nc.tensor.matmul(psn[:], bhhn_sb[:, l * H:(l + 1) * H], selq_sb[:],
                             start=True, stop=False, skip_group_check=True)

            if l == 0:
                x_ch = rhsp.tile([DIN + 2, 2, W], BF, tag="x")
                nc.sync.dma_start(x_ch[:], xfr[:, ds(2 * cw, 2 * W)])
                for dd in range(2):
                    src = x_ch[:, dd, :]
                    b0 = dd * 3 * H
                    nc.tensor.matmul(rz[:, dd, :], wih0_sb[:, b0:b0 + H],
                                     src, start=(dd == 0), stop=False,
                                     skip_group_check=True)
                    nc.tensor.matmul(rz[:, 2 + dd, :], wih0_sb[:, b0 + H:b0 + 2 * H],
                                     src, start=(dd == 0), stop=False,
                                     skip_group_check=True)
                    nc.tensor.matmul(nj[:, dd, :], wih0_sb[:, b0 + 2 * H:b0 + 3 * H],
                                     src, start=(dd == 0), stop=(dd == 1),
                                     skip_group_check=True)
            else:
                st = rhsp.tile([H, 2 * FAT], BF, tag="st")
                nc.sync.dma_start(st[:], h0c[:, ds(cc, 1), :])
                mm = rhsp.tile([H, 2 * FAT], BF, tag="mm")
                nc.sync.dma_start(mm[:], h0c[:, ds(NSTEP - 1 - cc, 1), :])
                ff, bb = st[:, 0:FAT], st[:, FAT:2 * FAT]
                frv, brv = mm[:, 0:FAT], mm[:, FAT:2 * FAT]
                bene = rhsp.tile([4, 2 * W], BF, tag="bene")
                nc.sync.dma_start(bene[:], bene4[:, ds(2 * cw, 2 * W)])
                nc.tensor.matmul(rz[:, 0:2, :], bias1q_sb[:, 0:H], bene[:],
                                 start=True, stop=False, skip_group_check=True)
                nc.tensor.matmul(rz[:, 2:4, :], bias1q_sb[:, H:2 * H], bene[:],
                                 start=True, stop=False, skip_group_check=True)
                nc.tensor.matmul(nj[:], bias1q_sb[:, 2 * H:3 * H], bene[:],
                                 start=True, stop=False, skip_group_check=True)
                for dd, (rA, rB) in enumerate(((ff, brv), (frv, bb))):
                    b0 = dd * 2 * 3 * H
                    for g, dst in ((0, rz[:, dd, :]), (1, rz[:, 2 + dd, :])):
                        nc.tensor.matmul(dst, wih1_sb[:, b0 + g * H:b0 + (g + 1) * H],
                                         rA, start=False, stop=False,
                                         skip_group_check=True)
                        nc.tensor.matmul(dst, wih1_sb[:, b0 + 3 * H + g * H:b0 + 3 * H + (g + 1) * H],
                                         rB, start=False, stop=False,
                                         skip_group_check=True)
                    nc.tensor.matmul(nj[:, dd, :], wih1_sb[:, b0 + 2 * H:b0 + 3 * H],
                                     rA, start=False, stop=False,
                                     skip_group_check=True)
                    nc.tensor.matmul(nj[:, dd, :], wih1_sb[:, b0 + 3 * H + 2 * H:b0 + 3 * H + 3 * H],
                                     rB, start=False, stop=(dd == 1),
                                     skip_group_check=True)

            nc.scalar.copy(gxn[:], nj[:])

            for j in range(C):
                h_prev = prev_ring if j == 0 else ring[:, j - 1, :]
                if ABL == "nochain":
                    h_prev = hstate[:]
                emit_step(l, j, ring, h_prev, rz, psn, gxn)

            h_dst = h0c if l == 0 else h1c
            nc.sync.dma_start(h_dst[:, ds(cc, 1), :], ring[:, 0, :])
            return ring

        hints = (mybir.EngineType.PE, mybir.EngineType.DVE)
        outer = (tc.For_i(0, npass, 1, name="npass", hint_engines=hints)
                 if npass > 1 else nullcontext())
        with outer:
            rec = ExitStack()
            open_pools(rec)
            for l in range(2):
                nc.vector.memset(hstate[:], 0.0)
                with tc.For_i(0, NITER, 1, name=f"layer{l}", hint_engines=hints) as i:
                    prev = hstate[:]
                    for u in range(UNROLL):
                        ring = emit_chunk(l, i, u, prev)
                        prev = ring[:, C - 1, :]
                    nc.vector.tensor_copy(hstate[:], prev)
            rec.close()

            # ---- head: out[r, b], r = seg*SEGL + rr ----
            with tc.tile_pool(name="headp", bufs=3) as hp, \
                 tc.tile_pool(name="headps", bufs=2, space="PSUM") as hps:
                for s in range(G):
                    sbf = slice(s * Bc, (s + 1) * Bc)
                    sbb = slice(FAT + s * Bc, FAT + (s + 1) * Bc)
                    for k in range(SEGL // CH):
                        fch = hp.tile([H, CH, Bc], BF, tag="fch")
                        i0 = 2 * T + k * CH
                        nc.sync.dma_start(fch[:], h1c[:, i0:i0 + CH, sbf])
                        bch = hp.tile([H, CH, Bc], BF, tag="bch")
                        u0 = T + SEGL - (k + 1) * CH
                        nc.sync.dma_start(bch[:, ::-1, :], h1c[:, u0:u0 + CH, sbb])
                        pso = hps.tile([1, CH * Bc], F32, tag="pso")
                        nc.tensor.matmul(pso[:], wout_sb[:, 0:1], fch[:],
                                         start=True, stop=False, skip_group_check=True)
                        nc.tensor.matmul(pso[:], wout_sb[:, 1:2], bch[:],
                                         start=False, stop=True, skip_group_check=True)
                        osb = hp.tile([1, CH * Bc], F32, tag="osb")
                        nc.scalar.activation(osb[:], pso[:], AF.Identity,
                                             bias=bout_sb[0:1, 0:1])
                        o0 = (s * SEGL + k * CH) * Bc
                        nc.sync.dma_start(out_flat[o0:o0 + CH * Bc], osb[:])
        stack.close()

    nc.compile()
    return nc


_PROGRAM_CACHE = {}


def _get_program(npass=1):
    if npass not in _PROGRAM_CACHE:
        _PROGRAM_CACHE[npass] = build_program(npass)
    return _PROGRAM_CACHE[npass]


def _pack_host_inputs(inputs):
    """Build the per-core input maps from the full problem inputs."""
    from ml_dtypes import bfloat16
    x = np.asarray(inputs["x"], np.float32)

    # padded input over t in [-2T, S+2T): 7 x-rows + bias-enable + null-enable
    PADT = S + 4 * T
    xpad = np.zeros((DIN + 2, PADT, B), np.float32)
    xpad[:DIN, 2 * T:2 * T + S] = x.transpose(2, 1, 0)
    xpad[DIN, 2 * T:2 * T + S] = 1.0
    xpad[DIN + 1, :2 * T] = 1.0
    xpad[DIN + 1, 2 * T + S:] = 1.0

    def gT(w, g):
        return np.asarray(w, np.float32)[g * H:(g + 1) * H].T

    whhT = np.stack([
        gT(inputs[f"whh{l}{d}"], g)
        for l in range(2) for d in "fb" for g in range(3)
    ]).astype(bfloat16)

    wih0T = np.zeros((2, DIN + 2, 3 * H), np.float32)
    for di, d in enumerate("fb"):
        wih = np.asarray(inputs[f"wih0{d}"], np.float32)
        bih = np.asarray(inputs[f"bih0{d}"], np.float32)
        bhh = np.asarray(inputs[f"bhh0{d}"], np.float32)
        wih0T[di, :DIN] = wih.T
        bias = bih.copy()
        bias[:2 * H] += bhh[:2 * H]
        wih0T[di, DIN] = bias
        wih0T[di, DIN + 1, :2 * H] = NULL

    wih1T = np.zeros((2, 2, H, 3 * H), np.float32)
    bias1q = np.zeros((3, 4, H), np.float32)
    bhhnq = np.zeros((2, 2, H), np.float32)
    for di, d in enumerate("fb"):
        wih = np.asarray(inputs[f"wih1{d}"], np.float32)
        bih = np.asarray(inputs[f"bih1{d}"], np.float32)
        bhh = np.asarray(inputs[f"bhh1{d}"], np.float32)
        for k in range(2):
            for g in range(3):
                wih1T[di, k, :, g * H:(g + 1) * H] = \
                    wih[g * H:(g + 1) * H, k * H:(k + 1) * H].T
        for g in range(3):
            bias = bih[g * H:(g + 1) * H].copy()
            if g < 2:
                bias += bhh[g * H:(g + 1) * H]
            bias1q[g, 2 * di] = bias
            if g < 2:
                bias1q[g, 2 * di + 1] = NULL
        bhhnq[1, di] = bhh[2 * H:]
        bhh0 = np.asarray(inputs[f"bhh0{d}"], np.float32)
        bhhnq[0, di] = bhh0[2 * H:]

    selq = np.zeros((2, C * 2 * FAT), np.float32)
    cols = np.arange(C * 2 * FAT)
    selq[0, (cols % (2 * FAT)) < FAT] = 1.0
    selq[1, (cols % (2 * FAT)) >= FAT] = 1.0

    woutp = np.zeros((H, 2), np.float32)
    wout = np.asarray(inputs["wout"], np.float32)
    woutp[:, 0] = wout[0, :H]
    woutp[:, 1] = wout[0, H:]
    boutp = np.asarray(inputs["bout"], np.float32).reshape(1, 1)

    shared = dict(
        whhT=whhT,
        wih0T=wih0T.astype(bfloat16),
        wih1T=wih1T.astype(bfloat16),
        bias1q=bias1q.astype(bfloat16),
        bhhnq=bhhnq.astype(bfloat16),
        selq=selq.astype(bfloat16),
        woutp=woutp.astype(bfloat16),
        boutp=boutp,
    )

    in_maps = []
    for c in range(NCORES):
        arrf = np.zeros((DIN + 2, NSTEP, G, Bc), np.float32)
        arrr = np.zeros((DIN + 2, NSTEP, G, Bc), np.float32)
        for s in range(G):
            win = xpad[:, SEGL * (G * c + s):SEGL * (G * c + s) + NSTEP, :]
            arrf[:, :, s, :] = win
            arrr[:, :, s, :] = win[:, ::-1, :]
        xf_c = arrf.reshape(DIN + 2, NCH, W)
        xr_c = arrr.reshape(DIN + 2, NCH, W)
        xfr_c = np.stack([xf_c, xr_c], axis=2)  # [9, NCH, 2, W]
        bene = np.zeros((4, NCH, 2 * W), np.float32)
        bene[0:2, :, 0:W] = xf_c[DIN:DIN + 2]
        bene[2:4, :, W:2 * W] = xr_c[DIN:DIN + 2]
        in_maps.append(dict(
            shared,
            xfr=np.ascontiguousarray(xfr_c.reshape(DIN + 2, NCH * 2 * W)).astype(bfloat16),
            bene4=np.ascontiguousarray(bene.reshape(4, NCH * 2 * W)).astype(bfloat16),
        ))
    return in_maps


def _assemble_output(results) -> np.ndarray:
    outs = [r["out"] for r in results]  # each [G*SEGL, Bc]
    return np.concatenate(outs, axis=0).T.astype(np.float32)


def kernel(**inputs) -> np.ndarray:
    from concourse import bass_utils
    nc = _get_program()
    in_maps = _pack_host_inputs(inputs)
    res = bass_utils.run_bass_kernel_spmd(nc, in_maps, core_ids=list(range(NCORES)))
    return _assemble_output(res.results)


# revision 16
# speedup vs baseline: 3.5622x; 1.4051x over previous
"""Trainium2 Bass kernel for a 2-layer bidirectional GRU + linear head.

Problem: B=64, S=4096, D_IN=7, H=128, PyTorch gate order (r, z, n).

Sharding: SEQUENCE-parallel, 32 segments of 128 steps across 8 NeuronCores.
Each core owns 4 segments, organized as TWO instruction-interleaved STREAMS
of 2 segments each. Within a stream, both directions x both segments pack
into each instruction (G=2, FAT=128 cols/dir); across streams, every step's
ops are emitted interleaved (MMs_A, sig_A, MMs_B, sig_B, ..., tanh_A,
tanh_B, tail_A, tail_B) so that while stream A's serial chain waits on a
cross-engine handoff, the in-order engines execute stream B's ops. This
fills the ~55% engine idle the packed single-stream version shows.

Boundary correctness uses the GRU's fast forgetting (cold-start divergence
< 1e-4 within 16 steps, decaying further below the bf16 noise floor):
scans run a [seg_start-2T, seg_end+T) window (T=16); out-of-range steps are
"null steps" (r,z driven to sigmoid(-30)~=0, n-gate x-term 0) that reset h
to ~0 in one step, so edge segments start with exactly h=0 under a uniform
SPMD program.
"""

import os

import numpy as np

import concourse.bass as bass
import concourse.tile as tile
from concourse import bacc, mybir
from concourse.bass import ds

F32 = mybir.dt.float32
BF = mybir.dt.bfloat16
AF = mybir.ActivationFunctionType
ALU = mybir.AluOpType

H = 128
DIN = 7
B = 64
S = 4096
NCORES = 8

NSTR = 2                # instruction-interleaved streams per core
G = 2                   # segment chains packed per stream
SEGL = S // (NCORES * NSTR * G)  # 128 owned steps per segment
T = 16                  # burn-in halo steps
NSTEP = SEGL + 3 * T    # 176 scan steps per direction per layer
Bc = B                  # full batch per chain
FAT = G * Bc            # 128 packed cols per direction per stream
C = 2                   # fat-steps per chunk
W = C * FAT             # 256 chunk columns per direction
NCH = NSTEP // C        # 88 chunks per stream per layer
UNROLL = int(os.environ.get('UNR', '22'))  # chunk-pairs per For_i iteration
NITER = NCH // UNROLL
CH = 8                  # head steps per chunk
NULL = -30.0
ABL = os.environ.get("ABL", "")
TAIL = os.environ.get("TAIL", "sub")
UNR = int(os.environ.get("UNR", "11"))


def build_program(npass=1):
    nc = bacc.Bacc("TRN2", target_bir_lowering=False, debug=False)

    # ---- DRAM I/O (per-stream tensors carry suffix 0/1) ----
    xfr, bene4, h0f, h0b, h1f, h1b = [], [], [], [], [], []
    for q in range(NSTR):
        xfr.append(nc.dram_tensor(f"xfr{q}", [DIN + 2, NCH * 2 * W], BF,
                                  kind="ExternalInput").ap())
        bene4.append(nc.dram_tensor(f"bene{q}", [4, NCH * 2 * W], BF,
                                    kind="ExternalInput").ap())
    whhT = nc.dram_tensor("whhT", [12, H, H], BF, kind="ExternalInput").ap()
    wih0T = nc.dram_tensor("wih0T", [2, DIN + 2, 3 * H], BF, kind="ExternalInput").ap()
    wih1T = nc.dram_tensor("wih1T", [2, 2, H, 3 * H], BF, kind="ExternalInput").ap()
    bias1q = nc.dram_tensor("bias1q", [3, 4, H], BF, kind="ExternalInput").ap()
    bhhnq = nc.dram_tensor("bhhnq", [2, 2, H], BF, kind="ExternalInput").ap()
    selq = nc.dram_tensor("selq", [2, C * 2 * FAT], BF, kind="ExternalInput").ap()
    woutp = nc.dram_tensor("woutp", [H, 2], BF, kind="ExternalInput").ap()
    boutp = nc.dram_tensor("boutp", [1, 1], F32, kind="ExternalInput").ap()
    out = nc.dram_tensor("out", [NSTR * G * SEGL, Bc], F32, kind="ExternalOutput").ap()
    out_flat = out.rearrange("s b -> (s b)")

    with tile.TileContext(nc) as tc:
        from contextlib import ExitStack, nullcontext

        stack = ExitStack()
        consts = stack.enter_context(tc.tile_pool(name="consts", bufs=1))

        whh_sb = consts.tile([H, 12 * H], BF)
        for k in range(12):
            nc.sync.dma_start(whh_sb[:, k * H:(k + 1) * H], whhT[k])
        wih0_sb = consts.tile([DIN + 2, 2 * 3 * H], BF)
        for d in range(2):
            nc.sync.dma_start(wih0_sb[:, d * 3 * H:(d + 1) * 3 * H], wih0T[d])
        wih1_sb = consts.tile([H, 4 * 3 * H], BF)
        for d in range(2):
            for k in range(2):
                c0 = (d * 2 + k) * 3 * H
                nc.sync.dma_start(wih1_sb[:, c0:c0 + 3 * H], wih1T[d, k])
        bias1q_sb = consts.tile([4, 3 * H], BF)
        for g in range(3):
            nc.sync.dma_start(bias1q_sb[:, g * H:(g + 1) * H], bias1q[g])
        bhhn_sb = consts.tile([2, 2 * H], BF)
        for l in range(2):
            nc.sync.dma_start(bhhn_sb[:, l * H:(l + 1) * H], bhhnq[l])
        selq_sb = consts.tile([2, C * 2 * FAT], BF)
        nc.sync.dma_start(selq_sb[:], selq[:])
        wout_sb = consts.tile([H, 2], BF)
        nc.sync.dma_start(wout_sb[:], woutp[:])
        bout_sb = consts.tile([1, 1], F32)
        nc.sync.dma_start(bout_sb[:], boutp[:])
        hstate = [consts.tile([H, 2 * FAT], BF, name=f"hstate{q}")
                  for q in range(NSTR)]

        for q in range(NSTR):
            h0f.append(nc.dram_tensor(f"h0c{q}", [H, NSTEP, 2 * FAT], BF,
                                      kind="Internal").ap())
            h1f.append(nc.dram_tensor(f"h1c{q}", [H, NSTEP, 2 * FAT], BF,
                                      kind="Internal").ap())

        def whh(l, d, g):
            k = (l * 2 + d) * 3 + g
            return whh_sb[:, k * H:(k + 1) * H]

        pools = {}

        def open_pools(rec):
            pools["rhsp"] = rec.enter_context(tc.tile_pool(name="rhsp", bufs=4))
            pools["gxnp"] = rec.enter_context(tc.tile_pool(name="gxnp", bufs=4))
            pools["ringp"] = rec.enter_context(tc.tile_pool(name="ringp", bufs=4))
            pools["stepp"] = rec.enter_context(tc.tile_pool(name="stepp", bufs=3))
            pools["psp"] = rec.enter_context(tc.tile_pool(name="psp", bufs=2, space="PSUM"))
            pools["psnp"] = rec.enter_context(tc.tile_pool(name="psnp", bufs=2, space="PSUM"))
            pools["njp"] = rec.enter_context(tc.tile_pool(name="njp", bufs=2, space="PSUM"))

        def emit_p1(l, j, st):
            """Step matmuls + merged sigmoid for one stream."""
            stepp = pools["stepp"]
            q = st["q"]
            rz, psn = st["rz"], st["psn"]
            js = slice(j * FAT, (j + 1) * FAT)
            h_prev = st["prev"]
            hf, hb = h_prev[:, 0:FAT], h_prev[:, FAT:2 * FAT]
            last = j == C - 1
            nc.tensor.matmul(rz[:, 0, js], whh(l, 0, 0), hf,
                             start=False, stop=False, skip_group_check=True)
            nc.tensor.matmul(rz[:, 1, js], whh(l, 1, 0), hb,
                             start=False, stop=last, skip_group_check=True)
            nc.tensor.matmul(psn[:, j, 0:FAT], whh(l, 0, 2), hf,
                             start=False, stop=False, skip_group_check=True)
            nc.tensor.matmul(psn[:, j, FAT:2 * FAT], whh(l, 1, 2), hb,
                             start=False, stop=last, skip_group_check=True)
            nc.tensor.matmul(rz[:, 2, js], whh(l, 0, 1), hf,
                             start=False, stop=False, skip_group_check=True)
            nc.tensor.matmul(rz[:, 3, js], whh(l, 1, 1), hb,
                             start=False, stop=last, skip_group_check=True)
            st["srz"] = stepp.tile([H, 4, FAT], BF, tag=f"srz{q}", name=f"srz{q}")
            nc.scalar.activation(st["srz"][:], rz[:, :, js], AF.Sigmoid)

        def emit_p2(j, st):
            """rn, arg (DVE) + tanh (ACT) for one stream."""
            stepp = pools["stepp"]
            q = st["q"]
            rn = stepp.tile([H, 2 * FAT], BF, tag=f"rn{q}", name=f"rn{q}")
            nc.vector.tensor_mul(rn[:], st["srz"][:, 0:2, :], st["psn"][:, j, :])
            arg = stepp.tile([H, 2 * FAT], BF, tag=f"arg{q}", name=f"arg{q}")
            nc.vector.tensor_add(arg[:], rn[:], st["gxn"][:, :, slice(j * FAT, (j + 1) * FAT)])
            st["n_t"] = stepp.tile([H, 2 * FAT], BF, tag=f"n{q}", name=f"n{q}")
            if TAIL == "omz":
                omz = stepp.tile([H, 2 * FAT], BF, tag=f"omz{q}", name=f"omz{q}")
                nc.vector.tensor_scalar(omz[:], st["srz"][:, 2:4, :], -1.0, 1.0,
                                        ALU.mult, ALU.add)
                st["omz"] = omz
                zh = stepp.tile([H, 2 * FAT], BF, tag=f"zh{q}", name=f"zh{q}")
                nc.vector.tensor_mul(zh[:], st["srz"][:, 2:4, :], st["prev"])
                st["zh"] = zh
            nc.scalar.activation(st["n_t"][:], arg[:], AF.Tanh)

        def emit_p3(j, st):
            stepp = pools["stepp"]
            q = st["q"]
            if TAIL == "omz":
                # h' = (1-z)*n + z*h; omz/zh were computed during tanh
                omzn = stepp.tile([H, 2 * FAT], BF, tag=f"omzn{q}", name=f"omzn{q}")
                nc.vector.tensor_mul(omzn[:], st["omz"][:], st["n_t"][:])
                nc.vector.tensor_add(st["ring"][:, j, :], omzn[:], st["zh"][:])
            else:
                # h' = n + z*(h-n)
                d_t = stepp.tile([H, 2 * FAT], BF, tag=f"d{q}", name=f"d{q}")
                nc.vector.tensor_sub(d_t[:], st["prev"], st["n_t"][:])
                zd = stepp.tile([H, 2 * FAT], BF, tag=f"zd{q}", name=f"zd{q}")
                nc.vector.tensor_mul(zd[:], st["srz"][:, 2:4, :], d_t[:])
                nc.vector.tensor_add(st["ring"][:, j, :], st["n_t"][:], zd[:])
            st["prev"] = hstate[q][:] if ABL == "nochain" else st["ring"][:, j, :]

        def emit_chunk_head(l, i, k, st):
            """Per-chunk tile alloc, loads, bulk gx matmuls for one stream."""
            q = st["q"]
            rhsp = pools["rhsp"]
            cw = (i * UNROLL + k) * W
            cc = (i * UNROLL + k) * C
            st["cc"] = cc
            rz = st["rz"] = pools["psp"].tile([H, 4, W], F32, tag="rz", name=f"rz{q}")
            psn = st["psn"] = pools["psnp"].tile([H, C, 2 * FAT], F32, tag="psn", name=f"psn{q}")
            nj = st["nj"] = pools["njp"].tile([H, 2, W], F32, tag="nj", name=f"nj{q}")
            gxn = st["gxn"] = pools["gxnp"].tile([H, 2, W], BF, tag="gxn", name=f"gxn{q}")
            st["ring"] = pools["ringp"].tile([H, C, 2 * FAT], BF, tag="ring", name=f"ring{q}")

            nc.tensor.matmul(psn[:], bhhn_sb[:, l * H:(l + 1) * H], selq_sb[:],
                             start=True, stop=False, skip_group_check=True)
            if l == 0:
                if k % 2 == 0:
                    st["xp"] = rhsp.tile([DIN + 2, 2, 2, W], BF,
                                         tag=f"x{q}", name=f"x{q}")
                    nc.sync.dma_start(st["xp"][:], xfr[q][:, ds(2 * cw, 4 * W)])
                for dd in range(2):
                    src = st["xp"][:, k % 2, dd, :]
                    b0 = dd * 3 * H
                    nc.tensor.matmul(rz[:, dd, :], wih0_sb[:, b0:b0 + H],
                                     src, start=(dd == 0), stop=False,
                                     skip_group_check=True)
                    nc.tensor.matmul(rz[:, 2 + dd, :], wih0_sb[:, b0 + H:b0 + 2 * H],
                                     src, start=(dd == 0), stop=False,
                                     skip_group_check=True)
                    nc.tensor.matmul(nj[:, dd, :], wih0_sb[:, b0 + 2 * H:b0 + 3 * H],
                                     src, start=(dd == 0), stop=(dd == 1),
                                     skip_group_check=True)
            else:
                if k % 2 == 0:
                    st["st4"] = rhsp.tile([H, 2 * C, 2 * FAT], BF,
                                          tag=f"st{q}", name=f"st{q}")
                    nc.sync.dma_start(st["st4"][:], h0f[q][:, ds(cc, 2 * C), :])
                    st["mm4"] = rhsp.tile([H, 2 * C, 2 * FAT], BF,
                                          tag=f"mm{q}", name=f"mm{q}")
                    nc.sync.dma_start(st["mm4"][:, ::-1, :],
                                      h0f[q][:, ds(NSTEP - 2 * C - cc, 2 * C), :])
                    st["bp"] = rhsp.tile([4, 2, 2 * W], BF,
                                         tag=f"be{q}", name=f"be{q}")
                    nc.sync.dma_start(st["bp"][:], bene4[q][:, ds(2 * cw, 4 * W)])
                h = (k % 2) * C
                stt = st["st4"][:, h:h + C, :]
                mmt = st["mm4"][:, h:h + C, :]
                ff, bb = stt[:, :, 0:FAT], stt[:, :, FAT:2 * FAT]
                frv, brv = mmt[:, :, 0:FAT], mmt[:, :, FAT:2 * FAT]
                bene = st["bp"][:, k % 2, :]
                nc.tensor.matmul(rz[:, 0:2, :], bias1q_sb[:, 0:H], bene,
                                 start=True, stop=False, skip_group_check=True)
                nc.tensor.matmul(rz[:, 2:4, :], bias1q_sb[:, H:2 * H], bene,
                                 start=True, stop=False, skip_group_check=True)
                nc.tensor.matmul(nj[:], bias1q_sb[:, 2 * H:3 * H], bene,
                                 start=True, stop=False, skip_group_check=True)
                for dd, (rA, rB) in enumerate(((ff, brv), (frv, bb))):
                    b0 = dd * 2 * 3 * H
                    for g, dst in ((0, rz[:, dd, :]), (1, rz[:, 2 + dd, :])):
                        nc.tensor.matmul(dst, wih1_sb[:, b0 + g * H:b0 + (g + 1) * H],
                                         rA, start=False, stop=False,
                                         skip_group_check=True)
                        nc.tensor.matmul(dst, wih1_sb[:, b0 + 3 * H + g * H:b0 + 3 * H + (g + 1) * H],
                                         rB, start=False, stop=False,
                                         skip_group_check=True)
                    nc.tensor.matmul(nj[:, dd, :], wih1_sb[:, b0 + 2 * H:b0 + 3 * H],
                                     rA, start=False, stop=False,
                                     skip_group_check=True)
                    nc.tensor.matmul(nj[:, dd, :], wih1_sb[:, b0 + 3 * H + 2 * H:b0 + 3 * H + 3 * H],
                                     rB, start=False, stop=(dd == 1),
                                     skip_group_check=True)
            if q == 0:
                nc.scalar.copy(gxn[:], nj[:])
            else:
                nc.vector.tensor_copy(gxn[:], nj[:])

        def emit_pair(l, i, k, streams):
            for st in streams:
                emit_chunk_head(l, i, k, st)
            for j in range(C):
                for st in streams:
                    emit_p1(l, j, st)
                for st in streams:
                    emit_p2(j, st)
                for st in streams:
                    emit_p3(j, st)
            for st in streams:
                q, cc, ring = st["q"], st["cc"], st["ring"]
                hdst = (h0f if l == 0 else h1f)[q]
                nc.sync.dma_start(hdst[:, ds(cc, C), :], ring[:])

        hints = (mybir.EngineType.PE, mybir.EngineType.DVE)
        outer = (tc.For_i(0, npass, 1, name="npass", hint_engines=hints)
                 if npass > 1 else nullcontext())
        with outer:
            rec = ExitStack()
            open_pools(rec)
            for l in range(2):
                streams = [{"q": q} for q in range(NSTR)]
                for q in range(NSTR):
                    nc.vector.memset(hstate[q][:], 0.0)
                with tc.For_i(0, NITER, 1, name=f"layer{l}", hint_engines=hints) as i:
                    for st in streams:
                        st["prev"] = hstate[st["q"]][:]
                    for k in range(UNROLL):
                        emit_pair(l, i, k, streams)
                    for st in streams:
                        nc.vector.tensor_copy(hstate[st["q"]][:], st["prev"])
            rec.close()

            # ---- head: out row = q*G*SEGL + s*SEGL + rr, 4 chunks/DMA group ----
            with tc.tile_pool(name="headp", bufs=3) as hp, \
                 tc.tile_pool(name="headps", bufs=2, space="PSUM") as hps:
                for q in range(NSTR):
                    for s in range(G):
                        sbf = slice(s * Bc, (s + 1) * Bc)
                        sbb = slice(FAT + s * Bc, FAT + (s + 1) * Bc)
                        for g4 in range(SEGL // CH // 4):
                            k0 = 4 * g4
                            fch = hp.tile([H, 4 * CH, Bc], BF, tag="fch")
                            i0 = 2 * T + k0 * CH
                            nc.sync.dma_start(fch[:], h1f[q][:, i0:i0 + 4 * CH, sbf])
                            bch = hp.tile([H, 4 * CH, Bc], BF, tag="bch")
                            u0 = T + SEGL - (k0 + 4) * CH
                            nc.sync.dma_start(bch[:, ::-1, :],
                                              h1f[q][:, u0:u0 + 4 * CH, sbb])
                            osb = hp.tile([1, 4 * CH * Bc], F32, tag="osb")
                            for p in range(4):
                                ps = slice(p * CH, (p + 1) * CH)
                                pso = hps.tile([1, CH * Bc], F32, tag="pso")
                                nc.tensor.matmul(pso[:], wout_sb[:, 0:1], fch[:, ps, :],
                                                 start=True, stop=False,
                                                 skip_group_check=True)
                                nc.tensor.matmul(pso[:], wout_sb[:, 1:2], bch[:, ps, :],
                                                 start=False, stop=True,
                                                 skip_group_check=True)
                                nc.scalar.activation(
                                    osb[0:1, p * CH * Bc:(p + 1) * CH * Bc],
                                    pso[:], AF.Identity, bias=bout_sb[0:1, 0:1])
                            o0 = ((q * G + s) * SEGL + k0 * CH) * Bc
                            nc.sync.dma_start(out_flat[o0:o0 + 4 * CH * Bc], osb[:])
        stack.close()

    nc.compile()
    return nc


_PROGRAM_CACHE = {}


def _get_program(npass=1):
    if npass not in _PROGRAM_CACHE:
        _PROGRAM_CACHE[npass] = build_program(npass)
    return _PROGRAM_CACHE[npass]


def _pack_host_inputs(inputs):
    from ml_dtypes import bfloat16
    x = np.asarray(inputs["x"], np.float32)

    PADT = S + 4 * T
    xpad = np.zeros((DIN + 2, PADT, B), np.float32)
    xpad[:DIN, 2 * T:2 * T + S] = x.transpose(2, 1, 0)
    xpad[DIN, 2 * T:2 * T + S] = 1.0
    xpad[DIN + 1, :2 * T] = 1.0
    xpad[DIN + 1, 2 * T + S:] = 1.0

    def gT(w, g):
        return np.asarray(w, np.float32)[g * H:(g + 1) * H].T

    whhT = np.stack([
        gT(inputs[f"whh{l}{d}"], g)
        for l in range(2) for d in "fb" for g in range(3)
    ]).astype(bfloat16)

    wih0T = np.zeros((2, DIN + 2, 3 * H), np.float32)
    for di, d in enumerate("fb"):
        wih = np.asarray(inputs[f"wih0{d}"], np.float32)
        bih = np.asarray(inputs[f"bih0{d}"], np.float32)
        bhh = np.asarray(inputs[f"bhh0{d}"], np.float32)
        wih0T[di, :DIN] = wih.T
        bias = bih.copy()
        bias[:2 * H] += bhh[:2 * H]
        wih0T[di, DIN] = bias
        wih0T[di, DIN + 1, :2 * H] = NULL

    wih1T = np.zeros((2, 2, H, 3 * H), np.float32)
    bias1q = np.zeros((3, 4, H), np.float32)
    bhhnq = np.zeros((2, 2, H), np.float32)
    for di, d in enumerate("fb"):
        wih = np.asarray(inputs[f"wih1{d}"], np.float32)
        bih = np.asarray(inputs[f"bih1{d}"], np.float32)
        bhh = np.asarray(inputs[f"bhh1{d}"], np.float32)
        for k in range(2):
            for g in range(3):
                wih1T[di, k, :, g * H:(g + 1) * H] = \
                    wih[g * H:(g + 1) * H, k * H:(k + 1) * H].T
        for g in range(3):
            bias = bih[g * H:(g + 1) * H].copy()
            if g < 2:
                bias += bhh[g * H:(g + 1) * H]
            bias1q[g, 2 * di] = bias
            if g < 2:
                bias1q[g, 2 * di + 1] = NULL
        bhhnq[1, di] = bhh[2 * H:]
        bhh0 = np.asarray(inputs[f"bhh0{d}"], np.float32)
        bhhnq[0, di] = bhh0[2 * H:]

    selq = np.zeros((2, C * 2 * FAT), np.float32)
    cols = np.arange(C * 2 * FAT)
    selq[0, (cols % (2 * FAT)) < FAT] = 1.0
    selq[1, (cols % (2 * FAT)) >= FAT] = 1.0

    woutp = np.zeros((H, 2), np.float32)
    wout = np.asarray(inputs["wout"], np.float32)
    woutp[:, 0] = wout[0, :H]
    woutp[:, 1] = wout[0, H:]
    boutp = np.asarray(inputs["bout"], np.float32).reshape(1, 1)

    shared = dict(
        whhT=whhT,
        wih0T=wih0T.astype(bfloat16),
        wih1T=wih1T.astype(bfloat16),
        bias1q=bias1q.astype(bfloat16),
        bhhnq=bhhnq.astype(bfloat16),
        selq=selq.astype(bfloat16),
        woutp=woutp.astype(bfloat16),
        boutp=boutp,
    )

    in_maps = []
    for c in range(NCORES):
        m = dict(shared)
        for q in range(NSTR):
            arrf = np.zeros((DIN + 2, NSTEP, G, Bc), np.float32)
            arrr = np.zeros((DIN + 2, NSTEP, G, Bc), np.float32)
            for s in range(G):
                gseg = (c * NSTR + q) * G + s
                win = xpad[:, SEGL * gseg:SEGL * gseg + NSTEP, :]
                arrf[:, :, s, :] = win
                arrr[:, :, s, :] = win[:, ::-1, :]
            xf_c = arrf.reshape(DIN + 2, NCH, W)
            xr_c = arrr.reshape(DIN + 2, NCH, W)
            xfr_c = np.stack([xf_c, xr_c], axis=2)  # [9, NCH, 2, W]
            bene = np.zeros((4, NCH, 2 * W), np.float32)
            bene[0:2, :, 0:W] = xf_c[DIN:DIN + 2]
            bene[2:4, :, W:2 * W] = xr_c[DIN:DIN + 2]
            m[f"xfr{q}"] = np.ascontiguousarray(
                xfr_c.reshape(DIN + 2, NCH * 2 * W)).astype(bfloat16)
            m[f"bene{q}"] = np.ascontiguousarray(
                bene.reshape(4, NCH * 2 * W)).astype(bfloat16)
        in_maps.append(m)
    return in_maps


def _assemble_output(results) -> np.ndarray:
    outs = [r["out"] for r in results]  # each [NSTR*G*SEGL, Bc]
    return np.concatenate(outs, axis=0).T.astype(np.float32)


def kernel(**inputs) -> np.ndarray:
    from concourse import bass_utils
    nc = _get_program()
    in_maps = _pack_host_inputs(inputs)
    res = bass_utils.run_bass_kernel_spmd(nc, in_maps, core_ids=list(range(NCORES)))
    return _assemble_output(res.results)
